# revision 1
# baseline (speedup 1.0000x reference)
"""Trainium2 Bass kernel for nn_Dynamic_7x7_naivev2 (CSPN-style propagation).

Self-contained: shards the batch x H-halves across 8 NeuronCores with an
18-row shrinking halo (no inter-core communication), runs a Bass/Tile
kernel per core, and reassembles the full output.
"""
import copy

import numpy as np

import bass_rust
import concourse.bass as bass
import concourse.mybir as mybir
from concourse.bass_utils import run_bass_kernel_spmd
from concourse.tile import TileContext
from contextlib import ExitStack






AF = mybir.ActivationFunctionType

R = 258          # local rows per shard
RPAD = 272       # padded DRAM rows for g/fi: 3 zero + 258 data + 11 zero
W = 640
X = 648          # q/feat tile cols (3 zero margin each side + 2 pad)
NT = 3           # row tiles
TSTEP = 122      # output rows per tile
CH = 48
XC = 320         # x chunk width (psum free dim)
CHUNKS = (0, 320)  # output col bases (global cols)

# (dy, dx) per guidance channel, ring 0 = 3x3 (ch 0:8), 1 = 5x5 (8:24),
# 2 = 7x7 (24:48). Derived numerically from the reference conv.
OFFS = [(1, 1), (1, 0), (1, -1), (0, 1), (0, -1), (-1, 1), (-1, 0), (-1, -1),
        (2, 2), (2, 1), (2, 0), (2, -1), (2, -2), (1, 2), (1, -2), (0, 2),
        (0, -2), (-1, 2), (-1, -2), (-2, 2), (-2, 1), (-2, 0), (-2, -1),
        (-2, -2),
        (3, 3), (3, 2), (3, 1), (3, 0), (3, -1), (3, -2), (3, -3), (2, 3),
        (2, -3), (1, 3), (1, -3), (0, 3), (0, -3), (-1, 3), (-1, -3),
        (-2, 3), (-2, -3), (-3, 3), (-3, 2), (-3, 1), (-3, 0), (-3, -1),
        (-3, -2), (-3, -3)]
RING_RANGES = ((0, 8), (8, 24), (24, 48))


def smat_np(qdt_np):
    """S matrices [7, 128, 122]; S[dy+3][k, j] = 1 iff k == j + dy + 3."""
    s = np.zeros((7, 128, TSTEP), dtype=np.float32)
    for dyi, dy in enumerate(range(-3, 4)):
        for j in range(TSTEP):
            k = j + dy + 3
            if 0 <= k < 128:
                s[dyi, k, j] = 1.0
    return s.astype(qdt_np)


def tile_geom(t):
    """(base_row, first_valid_part, end_valid_part, q_extent, valid_out)"""
    base = TSTEP * t - 3
    lo = max(0, -base)
    hi = min(128, R - base)
    qhi = min(128, hi + 7)
    vt = min(TSTEP, R - TSTEP * t)
    return base, lo, hi, qhi, vt


def act_recip(nc, out, in_):
    """scalar-engine Reciprocal, bypassing the accuracy guard (we Newton-refine)."""
    eng = nc.scalar
    return eng.add_instruction(
        mybir.InstActivation(
            name=nc.get_next_instruction_name(),
            func=AF.Reciprocal,
            ins=[eng.lower_ap(in_),
                 mybir.ImmediateValue(dtype=mybir.dt.float32, value=0.0),
                 mybir.ImmediateValue(dtype=mybir.dt.float32, value=1.0),
                 mybir.ImmediateValue(dtype=mybir.dt.float32, value=0.0)],
            outs=[eng.lower_ap(out)],
        )
    )


def build_nc(prop_time=6, qdt=mybir.dt.bfloat16):
    nc = bass.Bass()
    f32 = mybir.dt.float32

    g_in = nc.declare_dram_parameter("g", [CH, RPAD, W], f32, isOutput=False)
    dyn_in = nc.declare_dram_parameter("dyn", [4 * prop_time, R, W], f32,
                                       isOutput=False)
    fi_in = nc.declare_dram_parameter("fi", [RPAD, W], f32, isOutput=False)
    cf_in = nc.declare_dram_parameter("cf", [R, W], f32, isOutput=False)
    ff_in = nc.declare_dram_parameter("ff", [R, W], f32, isOutput=False)
    sm_in = nc.declare_dram_parameter("smat", [7, 128, TSTEP], qdt,
                                      isOutput=False)
    out = nc.declare_dram_parameter("out", [R, W], f32, isOutput=True)

    with ExitStack() as ctx:
        tc = ctx.enter_context(TileContext(nc))
        pool = ctx.enter_context(tc.tile_pool(name="main", bufs=1))
        pspool = ctx.enter_context(
            tc.tile_pool(name="ps", bufs=1, space="PSUM"))

        # ---- fixed tiles ----
        S = [pool.tile([128, TSTEP], qdt, tag=f"S{i}", name=f"S{i}") for i in range(7)]
        for i in range(7):
            nc.sync.dma_start(out=S[i][:], in_=sm_in[i])

        ft = [pool.tile([128, X], f32, tag=f"ft{t}", name=f"ft{t}") for t in range(NT)]
        fi_out = [pool.tile([TSTEP, W], f32, tag=f"fio{t}", name=f"fio{t}") for t in range(NT)]
        OM = [pool.tile([TSTEP, W], f32, tag=f"om{t}", name=f"om{t}") for t in range(NT)]
        FF = [pool.tile([TSTEP, W], f32, tag=f"ffp{t}", name=f"ffp{t}") for t in range(NT)]
        A = [[pool.tile([TSTEP, W], f32, tag=f"A{r}{t}", name=f"A{r}{t}") for t in range(NT)]
             for r in range(3)]
        D = [[pool.tile([TSTEP, W], f32, tag=f"D{r}{t}", name=f"D{r}{t}") for t in range(NT)]
             for r in range(3)]

        NG = 4
        NQ = 3
        gb = [pool.tile([128, W], f32, tag=f"gb{i}", name=f"gb{i}") for i in range(NG)]
        qb = [pool.tile([128, X], qdt, tag=f"qb{i}", name=f"qb{i}") for i in range(NQ)]
        fco = [pool.tile([TSTEP, W], f32, tag=f"fco{i}", name=f"fco{i}") for i in range(2)]
        dynb = [pool.tile([TSTEP, 4 * W], f32, tag=f"dynb{i}", name=f"dynb{i}")
                for i in range(2)]
        attb = [pool.tile([TSTEP, 4 * W], f32, tag=f"attb{i}", name=f"attb{i}")
                for i in range(2)]
        cfb = pool.tile([TSTEP, W], f32, tag="cfb", name="cfb")
        ffb = pool.tile([TSTEP, W], f32, tag="ffb", name="ffb")
        sgn = pool.tile([TSTEP, W], f32, tag="sgn", name="sgn")
        fxb = pool.tile([TSTEP, W], f32, tag="fxb", name="fxb")
        tmp_out = [pool.tile([TSTEP, XC], f32, tag=f"tout{i}", name=f"tout{i}")
                   for i in range(2)]
        NE = 8
        eb = [pool.tile([TSTEP, XC], f32, tag=f"eb{i}", name=f"eb{i}") for i in range(NE)]

        for t in range(NT):
            nc.vector.memset(ft[t][:], 0.0)
        for i in range(NQ):
            nc.vector.memset(qb[i][:], 0.0)

        def load_plane(dst, src, t):
            _, _, _, _, vt = tile_geom(t)
            r0 = TSTEP * t
            nc.sync.dma_start(out=dst[0:vt, :], in_=src[r0:r0 + vt, :])

        def load_g_tile(dst, ch, t, dram=g_in):
            """One DMA from the zero-padded DRAM plane: partition p of
            tile t <-> padded row 122t + p (= local row 122t - 3 + p)."""
            _, _, _, qhi, _ = tile_geom(t)
            nc.sync.dma_start(out=dst[0:qhi, :],
                              in_=dram[ch, TSTEP * t:TSTEP * t + qhi, :])

        def psum_tiles():
            return [[pspool.tile([TSTEP, XC], f32, tag=f"ps{r}{c}", name=f"ps{r}{c}")
                     for c in range(2)] for r in range(3)]

        def ring_sweep(t, ps, prep):
            """48-channel sweep: load g, prep(qq, g, qhi), then the ring
            shift-matmuls of qq into ps[ring][chunk]."""
            base, lo, hi, qhi, vt = tile_geom(t)
            for ri, (c0, c1) in enumerate(RING_RANGES):
                for ch in range(c0, c1):
                    g = gb[ch % NG]
                    load_g_tile(g, ch, t)
                    qq = qb[ch % NQ]
                    prep(qq, g, qhi)
                    dy, dx = OFFS[ch]
                    first = ch == c0
                    last = ch == c1 - 1
                    for ci, cb in enumerate(CHUNKS):
                        nc.tensor.matmul(
                            ps[ri][ci][:],
                            lhsT=S[dy + 3][:],
                            rhs=qq[:, cb + 3 + dx:cb + 3 + dx + XC],
                            start=first, stop=last)

        # ================= setup =================
        for t in range(NT):
            base, lo, hi, qhi, vt = tile_geom(t)
            n = min(128, R + 3 - TSTEP * t)
            nc.sync.dma_start(out=ft[t][0:n, 3:3 + W],
                              in_=fi_in[TSTEP * t:TSTEP * t + n, :])
            r0 = TSTEP * t
            nc.sync.dma_start(out=fi_out[t][0:vt, :],
                              in_=fi_in[r0 + 3:r0 + 3 + vt, :])
            load_plane(cfb, cf_in, t)
            load_plane(ffb, ff_in, t)
            nc.scalar.sign(out=sgn[0:vt], in_=ffb[0:vt])
            nc.vector.tensor_mul(out=fxb[0:vt, :], in0=sgn[0:vt, :],
                                 in1=cfb[0:vt, :])
            nc.scalar.activation(out=OM[t][0:vt], in_=fxb[0:vt], func=AF.Copy,
                                 bias=1.0, scale=-1.0)
            nc.vector.tensor_mul(out=FF[t][0:vt, :], in0=fxb[0:vt, :],
                                 in1=ffb[0:vt, :])

        # aff sums: A = ring sums of |g| at output rows; D = A - sums of g
        for t in range(NT):
            base, lo, hi, qhi, vt = tile_geom(t)

            def prep_abs(qq, g, qh):
                nc.scalar.activation(out=qq[0:qh, 3:3 + W], in_=g[0:qh, :],
                                     func=AF.Abs)

            psA = psum_tiles()
            ring_sweep(t, psA, prep_abs)
            for ri in range(3):
                for ci, cb in enumerate(CHUNKS):
                    nc.scalar.copy(out=A[ri][t][0:vt, cb:cb + XC],
                                   in_=psA[ri][ci][0:vt, :])

            def prep_plain(qq, g, qh):
                nc.vector.tensor_copy(out=qq[0:qh, 3:3 + W], in_=g[0:qh, :])

            psB = psum_tiles()
            ring_sweep(t, psB, prep_plain)
            for ri in range(3):
                for ci, cb in enumerate(CHUNKS):
                    nc.vector.tensor_sub(out=D[ri][t][0:vt, cb:cb + XC],
                                         in0=A[ri][t][0:vt, cb:cb + XC],
                                         in1=psB[ri][ci][0:vt, :])

        # ================= iterations =================
        for it in range(prop_time):
            for t in range(NT):
                base, lo, hi, qhi, vt = tile_geom(t)
                fc = fco[t % 2]
                nc.sync.dma_start(out=fc[0:vt, :],
                                  in_=ft[t][3:3 + vt, 3:3 + W])
                dynt = dynb[t % 2]
                att = attb[t % 2]
                r0 = TSTEP * t
                for c in range(4):
                    nc.sync.dma_start(
                        out=dynt[0:vt, c * W:(c + 1) * W],
                        in_=dyn_in[4 * it + c, r0:r0 + vt, :])
                nc.scalar.activation(out=att[0:vt, :], in_=dynt[0:vt, :],
                                     func=AF.Sigmoid)

                def prep_mul(qq, g, qh, t=t):
                    nc.vector.tensor_mul(out=qq[0:qh, 3:3 + W],
                                         in0=ft[t][0:qh, 3:3 + W],
                                         in1=g[0:qh, :])

                ps = psum_tiles()
                ring_sweep(t, ps, prep_mul)

                for ci, cb in enumerate(CHUNKS):
                    a0 = att[0:vt, 0 * W + cb:0 * W + cb + XC]
                    a1 = att[0:vt, 1 * W + cb:1 * W + cb + XC]
                    a2 = att[0:vt, 2 * W + cb:2 * W + cb + XC]
                    a3 = att[0:vt, 3 * W + cb:3 * W + cb + XC]
                    u0, u1, u2, u3, u4, u5, u6, u7 = (
                        e[0:vt, :] for e in eb)
                    Ac = [A[r][t][0:vt, cb:cb + XC] for r in range(3)]
                    Dc = [D[r][t][0:vt, cb:cb + XC] for r in range(3)]
                    # e = a0*A0 + a1*A1 + a2*A2 + (a3 + 1e-4)
                    nc.vector.tensor_mul(out=u0, in0=a0, in1=Ac[0])
                    nc.vector.tensor_mul(out=u1, in0=a1, in1=Ac[1])
                    nc.vector.tensor_add(out=u0, in0=u0, in1=u1)
                    nc.vector.tensor_mul(out=u2, in0=a2, in1=Ac[2])
                    nc.vector.tensor_scalar_add(u3, a3, 1e-4)
                    nc.vector.tensor_add(out=u2, in0=u2, in1=u3)
                    nc.vector.tensor_add(out=u0, in0=u0, in1=u2)  # u0 = e
                    # d = a0*D0 + a1*D1 + a2*D2 + 1e-4
                    nc.vector.tensor_mul(out=u1, in0=a0, in1=Dc[0])
                    nc.vector.tensor_mul(out=u2, in0=a1, in1=Dc[1])
                    nc.vector.tensor_add(out=u1, in0=u1, in1=u2)
                    nc.vector.tensor_mul(out=u2, in0=a2, in1=Dc[2])
                    nc.vector.tensor_add(out=u1, in0=u1, in1=u2)
                    nc.vector.tensor_scalar_add(u2, u1, 1e-4)  # u2 = d
                    # num = a0*s3 + a1*s5 + a2*s7 + a3*feat + d*feat_init
                    nc.vector.tensor_mul(out=u3, in0=a0,
                                         in1=ps[0][ci][0:vt, :])
                    nc.vector.tensor_mul(out=u4, in0=a1,
                                         in1=ps[1][ci][0:vt, :])
                    nc.vector.tensor_add(out=u3, in0=u3, in1=u4)
                    nc.vector.tensor_mul(out=u4, in0=a2,
                                         in1=ps[2][ci][0:vt, :])
                    fc_c = fc[0:vt, cb:cb + XC]
                    nc.vector.tensor_mul(out=u5, in0=a3, in1=fc_c)
                    nc.vector.tensor_add(out=u4, in0=u4, in1=u5)
                    nc.vector.tensor_mul(out=u5, in0=u2,
                                         in1=fi_out[t][0:vt, cb:cb + XC])
                    nc.vector.tensor_add(out=u3, in0=u3, in1=u4)
                    nc.vector.tensor_add(out=u3, in0=u3, in1=u5)  # num
                    # r = 1/e: ACT table recip + one Newton step
                    act_recip(nc, u6, u0)
                    nc.vector.tensor_mul(out=u4, in0=u0, in1=u6)
                    nc.scalar.activation(out=u4, in_=u4, func=AF.Copy,
                                         bias=2.0, scale=-1.0)
                    nc.vector.tensor_mul(out=u6, in0=u6, in1=u4)
                    nc.vector.tensor_mul(out=u7, in0=u3, in1=u6)
                    to = tmp_out[ci]
                    nc.vector.tensor_mul(out=to[0:vt, :],
                                         in0=OM[t][0:vt, cb:cb + XC],
                                         in1=u7)
                    nc.vector.tensor_add(out=to[0:vt, :],
                                         in0=to[0:vt, :],
                                         in1=FF[t][0:vt, cb:cb + XC])
                    nc.sync.dma_start(
                        out=ft[t][3:3 + vt, 3 + cb:3 + cb + XC],
                        in_=to[0:vt, :])
            # seams between tiles (new feat values)
            nc.sync.dma_start(out=ft[1][0:3, :], in_=ft[0][122:125, :])
            nc.sync.dma_start(out=ft[0][125:128, :], in_=ft[1][3:6, :])
            nc.sync.dma_start(out=ft[2][0:3, :], in_=ft[1][122:125, :])
            nc.sync.dma_start(out=ft[1][125:128, :], in_=ft[2][3:6, :])

        # ================= output =================
        for t in range(NT):
            _, _, _, _, vt = tile_geom(t)
            r0 = TSTEP * t
            nc.sync.dma_start(out=out[r0:r0 + vt, :],
                              in_=ft[t][3:3 + vt, 3:3 + W])

    return nc


def fixup_waits(nc, cap=1):
    """Split >cap semaphore waits per instruction into prefix NoOps
    (this toolchain's codegen rejects multi-wait instructions)."""
    n_fixed = 0
    for f in nc.m.functions:
        for bb in f.blocks:
            insts = bb.instructions
            idx = 0
            changed = False
            while idx < len(insts):
                inst = insts[idx]
                si = inst.sync_info
                if si is None or si.on_wait is None or len(si.on_wait) <= cap:
                    idx += 1
                    continue
                waits = list(si.on_wait)
                head = waits[:-cap]
                for j in range(0, len(head), cap):
                    pre = bass_rust.InstNoOp(name=f"{inst.name}_wsplit{j}")
                    pre.engine = inst.engine
                    pre.debug = inst.debug
                    psi = copy.deepcopy(si)
                    psi.on_wait = head[j:j + cap]
                    psi.on_update = []
                    pre.sync_info = psi
                    insts.insert(idx, pre)
                    idx += 1
                si2 = inst.sync_info
                si2.on_wait = waits[-cap:]
                inst.sync_info = si2
                n_fixed += 1
                changed = True
                idx += 1
            if changed:
                bb.instructions = insts
    return n_fixed


_CACHE = {}


def _get_nc(qdt):
    if qdt not in _CACHE:
        nc = build_nc(prop_time=6, qdt=qdt)
        fixup_waits(nc)
        _CACHE[qdt] = nc
    return _CACHE[qdt]


def kernel(feat_init, guidance, dynamic, confidence, feat_fix,
           _trace=False, _qdt=None):
    qdt = _qdt or mybir.dt.bfloat16
    qdt_np = np.float32 if qdt == mybir.dt.float32 else None
    if qdt_np is None:
        import ml_dtypes
        qdt_np = ml_dtypes.bfloat16
    nc = _get_nc(qdt)
    B, _, H, Wf = feat_init.shape
    sm = smat_np(qdt_np)
    in_maps = []
    for c in range(8):
        b, half = c // 2, c % 2
        r0 = 0 if half == 0 else H - R
        rows = slice(r0, r0 + R)
        gpad = np.zeros((CH, RPAD, W), np.float32)
        gpad[:, 3:3 + R] = guidance[b, :, rows, :]
        fpad = np.zeros((RPAD, W), np.float32)
        fpad[3:3 + R] = feat_init[b, 0, rows, :]
        in_maps.append({
            "g": gpad,
            "dyn": np.ascontiguousarray(dynamic[b, :, rows, :],
                                        dtype=np.float32),
            "fi": fpad,
            "cf": np.ascontiguousarray(confidence[b, 0, rows, :],
                                       dtype=np.float32),
            "ff": np.ascontiguousarray(feat_fix[b, 0, rows, :],
                                       dtype=np.float32),
            "smat": sm,
        })
    try:
        res = run_bass_kernel_spmd(nc, in_maps, core_ids=list(range(8)),
                                   trace=_trace)
    except ModuleNotFoundError:
        # NTFF profiling hook unavailable in this environment
        res = run_bass_kernel_spmd(nc, in_maps, core_ids=list(range(8)),
                                   trace=False)
    outf = np.zeros((B, 1, H, W), np.float32)
    for c in range(8):
        b, half = c // 2, c % 2
        o = res.results[c]["out"]
        if half == 0:
            outf[b, 0, 0:240] = o[0:240]
        else:
            outf[b, 0, H - 240:H] = o[R - 240:R]
    if _trace:
        return outf, res
    return outf



# revision 36
# speedup vs baseline: 10.1837x; 10.1837x over previous
"""Trainium2 Bass kernel for nn_Dynamic_7x7_naivev2 (CSPN-style propagation).

Self-contained: shards the batch x H-halves across 8 NeuronCores with an
18-row shrinking halo (no inter-core communication), runs a Bass/Tile
kernel per core, and reassembles the full output.

guidance/dynamic ship as fp8(e3m4) in a single DRAM param and are
upconverted on-engine; fi/cf/ff ship as one stacked bf16 param; the S
shift matrices are generated on device; the boundary zero-halo is
materialized on device (per-tile pre-zeroed g buffers) instead of
host-side padded copies. All of this exists to minimize host->device
bytes and per-call host work, which dominate wall time here.
"""
import copy

import numpy as np

import jax

# Persistent XLA executable cache: the per-call pjit re-compile otherwise
# re-runs the NEFF verify/package step (~0.8 s) on every invocation.
jax.config.update("jax_compilation_cache_dir", "/tmp/jax_cache")
jax.config.update("jax_persistent_cache_min_compile_time_secs", 0)
jax.config.update("jax_persistent_cache_min_entry_size_bytes", 0)

import bass_rust
import concourse.bass as bass
import concourse.mybir as mybir
from concourse.bass_utils import run_bass_kernel_spmd
from concourse.tile import TileContext
from contextlib import ExitStack


AF = mybir.ActivationFunctionType

R = 258          # local rows per shard
RPAD = 272       # padded DRAM rows for fi: 3 zero + 258 data + 11 zero
W = 640
X = 648          # q/feat tile cols (3 zero margin each side + 2 pad)
NT = 3           # row tiles
TSTEP = 122      # output rows per tile
CH = 48
XC = 320         # x chunk width (psum free dim)
CHUNKS = (0, 320)  # output col bases (global cols)

# (dy, dx) per guidance channel, ring 0 = 3x3 (ch 0:8), 1 = 5x5 (8:24),
# 2 = 7x7 (24:48). Derived numerically from the reference conv.
OFFS = [(1, 1), (1, 0), (1, -1), (0, 1), (0, -1), (-1, 1), (-1, 0), (-1, -1),
        (2, 2), (2, 1), (2, 0), (2, -1), (2, -2), (1, 2), (1, -2), (0, 2),
        (0, -2), (-1, 2), (-1, -2), (-2, 2), (-2, 1), (-2, 0), (-2, -1),
        (-2, -2),
        (3, 3), (3, 2), (3, 1), (3, 0), (3, -1), (3, -2), (3, -3), (2, 3),
        (2, -3), (1, 3), (1, -3), (0, 3), (0, -3), (-1, 3), (-1, -3),
        (-2, 3), (-2, -3), (-3, 3), (-3, 2), (-3, 1), (-3, 0), (-3, -1),
        (-3, -2), (-3, -3)]
RING_RANGES = ((0, 8), (8, 24), (24, 48))


def dy_groups(c0, c1):
    """Channels of a ring grouped by row shift dy: [(dy, [(ch, dx), ...])]."""
    g = {}
    for ch in range(c0, c1):
        dy, dx = OFFS[ch]
        g.setdefault(dy, []).append((ch, dx))
    return sorted(g.items())


def tile_geom(t):
    """(base_row, first_valid_part, end_valid_part, q_extent, valid_out)"""
    base = TSTEP * t - 3
    lo = max(0, -base)
    hi = min(128, R - base)
    qhi = min(128, hi + 7)
    vt = min(TSTEP, R - TSTEP * t)
    return base, lo, hi, qhi, vt


def act_recip(nc, out, in_):
    """scalar-engine Reciprocal, bypassing the accuracy guard (we Newton-refine)."""
    eng = nc.scalar
    return eng.add_instruction(
        mybir.InstActivation(
            name=nc.get_next_instruction_name(),
            func=AF.Reciprocal,
            ins=[eng.lower_ap(in_),
                 mybir.ImmediateValue(dtype=mybir.dt.float32, value=0.0),
                 mybir.ImmediateValue(dtype=mybir.dt.float32, value=1.0),
                 mybir.ImmediateValue(dtype=mybir.dt.float32, value=0.0)],
            outs=[eng.lower_ap(out)],
        )
    )


def build_nc(prop_time=6, qdt=mybir.dt.bfloat16, gdt=mybir.dt.float8e3,
             ddt=mybir.dt.float8e3):
    nc = bass.Bass()
    f32 = mybir.dt.float32

    bf16 = mybir.dt.bfloat16
    assert gdt == ddt, "g and dyn share one DRAM param"
    # planes 0:48 = guidance channels, 48:72 = dynamic planes (4*it + c)
    g_in = nc.declare_dram_parameter("gd", [CH + 4 * prop_time, R, W], gdt,
                                     isOutput=False)
    dyn_plane = CH
    # rows 0:272 = fi (3-row zero pad + 258 + 11 pad), 272:530 = cf,
    # 530:788 = ff — one bf16 param, one host->device put.
    AUX_CF = RPAD
    AUX_FF = RPAD + R
    aux_in = nc.declare_dram_parameter("aux", [RPAD + 2 * R, W], bf16,
                                       isOutput=False)
    out = nc.declare_dram_parameter("out", [R, W], bf16, isOutput=True)

    with ExitStack() as ctx:
        tc = ctx.enter_context(TileContext(nc))
        pool = ctx.enter_context(tc.tile_pool(name="main", bufs=1))
        pspool = ctx.enter_context(
            tc.tile_pool(name="ps", bufs=1, space="PSUM"))

        # ---- fixed tiles ----
        # S[dyi][k, j] = 1 iff k == j + dyi — generated on device, no input.
        S = [pool.tile([128, TSTEP], qdt, tag=f"S{i}", name=f"S{i}") for i in range(7)]
        for i in range(7):
            nc.gpsimd.memset(S[i][:], 1.0)
            nc.gpsimd.affine_select(
                out=S[i][:], in_=S[i][:],
                compare_op=mybir.AluOpType.is_equal, fill=0.0,
                base=-i, channel_multiplier=1, pattern=[[-1, TSTEP]])

        ft = [pool.tile([128, X], f32, tag=f"ft{t}", name=f"ft{t}") for t in range(NT)]
        fi_out = [pool.tile([TSTEP, W], f32, tag=f"fio{t}", name=f"fio{t}") for t in range(NT)]
        OM = [pool.tile([TSTEP, W], f32, tag=f"om{t}", name=f"om{t}") for t in range(NT)]
        FF = [pool.tile([TSTEP, W], f32, tag=f"ffp{t}", name=f"ffp{t}") for t in range(NT)]
        A = [[pool.tile([TSTEP, W], f32, tag=f"A{r}{t}", name=f"A{r}{t}") for t in range(NT)]
             for r in range(3)]
        D = [[pool.tile([TSTEP, W], f32, tag=f"D{r}{t}", name=f"D{r}{t}") for t in range(NT)]
             for r in range(3)]

        NG = 6
        NQ = 4
        # per-tile g buffers: boundary partitions stay zero from the initial
        # memset (loads never touch them), giving the halo for free.
        gb = [[pool.tile([128, W], gdt, tag=f"gb{t}_{i}", name=f"gb{t}_{i}")
               for i in range(NG)] for t in range(NT)]
        qb = [pool.tile([128, X], qdt, tag=f"qb{i}", name=f"qb{i}") for i in range(NQ)]
        fco = [pool.tile([TSTEP, W], f32, tag=f"fco{i}", name=f"fco{i}") for i in range(2)]
        dynb = [pool.tile([TSTEP, 4 * W], ddt, tag=f"dynb{i}", name=f"dynb{i}")
                for i in range(2)]
        attb = [pool.tile([TSTEP, 4 * W], f32, tag=f"attb{i}", name=f"attb{i}")
                for i in range(2)]
        cfb = pool.tile([TSTEP, W], bf16, tag="cfb", name="cfb")
        ffb = pool.tile([TSTEP, W], bf16, tag="ffb", name="ffb")
        fstage = pool.tile([128, W], bf16, tag="fstage", name="fstage")
        fstage2 = pool.tile([TSTEP, W], bf16, tag="fstage2", name="fstage2")
        sgn = pool.tile([TSTEP, W], f32, tag="sgn", name="sgn")
        fxb = pool.tile([TSTEP, W], f32, tag="fxb", name="fxb")
        tmp_out = [pool.tile([TSTEP, XC], f32, tag=f"tout{i}", name=f"tout{i}")
                   for i in range(2)]
        NE = 8
        eb = [pool.tile([TSTEP, XC], f32, tag=f"eb{i}", name=f"eb{i}") for i in range(NE)]
        NU = 2
        ub = [pool.tile([128, X], qdt, tag=f"ub{i}", name=f"ub{i}") for i in range(NU)]
        u_ctr = [0]

        for t in range(NT):
            nc.vector.memset(ft[t][:], 0.0)
            for i in range(NG):
                nc.vector.memset(gb[t][i][:], 0.0)
        for i in range(NQ):
            nc.vector.memset(qb[i][:], 0.0)

        def load_plane(dst, aux_base, t):
            _, _, _, _, vt = tile_geom(t)
            r0 = aux_base + TSTEP * t
            nc.sync.dma_start(out=dst[0:vt, :], in_=aux_in[r0:r0 + vt, :])

        def load_g_tile(dst, ch, t):
            """partition p of tile t <-> local row 122t - 3 + p; rows outside
            [0, R) stay zero in dst from the init memset."""
            base, _, _, qhi, _ = tile_geom(t)
            p0 = max(0, -base)
            n = min(R, base + qhi) - (base + p0)
            nc.sync.dma_start(out=dst[p0:p0 + n, :],
                              in_=g_in[ch, base + p0:base + p0 + n, :])

        def psum_tiles():
            return [[pspool.tile([TSTEP, XC], f32, tag=f"ps{r}{c}", name=f"ps{r}{c}")
                     for c in range(2)] for r in range(3)]

        def ring_sweep(t, ps, prep):
            """48-channel sweep: load g, prep(qq, g, qhi), then the ring
            shift-matmuls of qq into ps[ring][chunk]."""
            base, lo, hi, qhi, vt = tile_geom(t)
            for ri, (c0, c1) in enumerate(RING_RANGES):
                for ch in range(c0, c1):
                    g = gb[t][ch % NG]
                    load_g_tile(g, ch, t)
                    qq = qb[ch % NQ]
                    prep(qq, g, qhi)
                    dy, dx = OFFS[ch]
                    first = ch == c0
                    last = ch == c1 - 1
                    for ci, cb in enumerate(CHUNKS):
                        nc.tensor.matmul(
                            ps[ri][ci][:],
                            lhsT=S[dy + 3][:],
                            rhs=qq[:, cb + 3 + dx:cb + 3 + dx + XC],
                            start=first, stop=last)

        def ring_sweep_grouped(t, ps, prep):
            """Like ring_sweep, but channels sharing a row shift dy are
            pre-summed (with their column shifts) on the vector engine into
            one u tile, so each ring needs one matmul pair per dy instead
            of per channel (96 -> ~30 PE instructions per sweep-tile)."""
            base, lo, hi, qhi, vt = tile_geom(t)
            for ri, (c0, c1) in enumerate(RING_RANGES):
                groups = dy_groups(c0, c1)
                for gi, (dy, members) in enumerate(groups):
                    u = ub[u_ctr[0] % NU]
                    u_ctr[0] += 1
                    for mi, (ch, dx) in enumerate(members):
                        g = gb[t][ch % NG]
                        load_g_tile(g, ch, t)
                        qq = qb[ch % NQ]
                        prep(qq, g, qhi)
                        if mi == 0:
                            nc.vector.tensor_copy(
                                out=u[0:qhi, 3:3 + W],
                                in_=qq[0:qhi, 3 + dx:3 + dx + W])
                        else:
                            nc.vector.tensor_add(
                                out=u[0:qhi, 3:3 + W],
                                in0=u[0:qhi, 3:3 + W],
                                in1=qq[0:qhi, 3 + dx:3 + dx + W])
                    first = gi == 0
                    last = gi == len(groups) - 1
                    for ci, cb in enumerate(CHUNKS):
                        nc.tensor.matmul(
                            ps[ri][ci][:],
                            lhsT=S[dy + 3][:],
                            rhs=u[:, cb + 3:cb + 3 + XC],
                            start=first, stop=last)

        # ================= setup =================
        for t in range(NT):
            base, lo, hi, qhi, vt = tile_geom(t)
            n = min(128, R + 3 - TSTEP * t)
            nc.sync.dma_start(out=fstage[0:n, :],
                              in_=aux_in[TSTEP * t:TSTEP * t + n, :])
            nc.vector.tensor_copy(out=ft[t][0:n, 3:3 + W],
                                  in_=fstage[0:n, :])
            r0 = TSTEP * t
            nc.sync.dma_start(out=fstage2[0:vt, :],
                              in_=aux_in[r0 + 3:r0 + 3 + vt, :])
            nc.vector.tensor_copy(out=fi_out[t][0:vt, :],
                                  in_=fstage2[0:vt, :])
            load_plane(cfb, AUX_CF, t)
            load_plane(ffb, AUX_FF, t)
            nc.scalar.sign(out=sgn[0:vt], in_=ffb[0:vt])
            nc.vector.tensor_mul(out=fxb[0:vt, :], in0=sgn[0:vt, :],
                                 in1=cfb[0:vt, :])
            nc.scalar.activation(out=OM[t][0:vt], in_=fxb[0:vt], func=AF.Copy,
                                 bias=1.0, scale=-1.0)
            nc.vector.tensor_mul(out=FF[t][0:vt, :], in0=fxb[0:vt, :],
                                 in1=ffb[0:vt, :])

        # aff sums at output rows: A = ring sums of |g|; D = A - sums of g
        # = ring sums of (|g| - g) = 2 * ring sums of relu(-g). Both are
        # sums of nonnegative terms, so the grouped (bf16-presummed) sweep
        # is cancellation-free for each.
        for t in range(NT):
            base, lo, hi, qhi, vt = tile_geom(t)

            def prep_abs(qq, g, qh):
                nc.scalar.activation(out=qq[0:qh, 3:3 + W], in_=g[0:qh, :],
                                     func=AF.Abs)

            psA = psum_tiles()
            ring_sweep_grouped(t, psA, prep_abs)
            for ri in range(3):
                for ci, cb in enumerate(CHUNKS):
                    nc.scalar.copy(out=A[ri][t][0:vt, cb:cb + XC],
                                   in_=psA[ri][ci][0:vt, :])

            def prep_negrelu(qq, g, qh):
                nc.scalar.activation(out=qq[0:qh, 3:3 + W], in_=g[0:qh, :],
                                     func=AF.Relu, scale=-1.0)

            psB = psum_tiles()
            ring_sweep_grouped(t, psB, prep_negrelu)
            for ri in range(3):
                for ci, cb in enumerate(CHUNKS):
                    nc.scalar.activation(out=D[ri][t][0:vt, cb:cb + XC],
                                         in_=psB[ri][ci][0:vt, :],
                                         func=AF.Copy, scale=2.0)

        # ================= iterations =================
        for it in range(prop_time):
            for t in range(NT):
                base, lo, hi, qhi, vt = tile_geom(t)
                fc = fco[t % 2]
                nc.sync.dma_start(out=fc[0:vt, :],
                                  in_=ft[t][3:3 + vt, 3:3 + W])
                dynt = dynb[t % 2]
                att = attb[t % 2]
                r0 = TSTEP * t
                for c in range(4):
                    nc.sync.dma_start(
                        out=dynt[0:vt, c * W:(c + 1) * W],
                        in_=g_in[dyn_plane + 4 * it + c, r0:r0 + vt, :])
                nc.scalar.activation(out=att[0:vt, :], in_=dynt[0:vt, :],
                                     func=AF.Sigmoid)

                def prep_mul(qq, g, qh, t=t):
                    nc.vector.tensor_mul(out=qq[0:qh, 3:3 + W],
                                         in0=ft[t][0:qh, 3:3 + W],
                                         in1=g[0:qh, :])

                ps = psum_tiles()
                ring_sweep_grouped(t, ps, prep_mul)

                for ci, cb in enumerate(CHUNKS):
                    a0 = att[0:vt, 0 * W + cb:0 * W + cb + XC]
                    a1 = att[0:vt, 1 * W + cb:1 * W + cb + XC]
                    a2 = att[0:vt, 2 * W + cb:2 * W + cb + XC]
                    a3 = att[0:vt, 3 * W + cb:3 * W + cb + XC]
                    u0, u1, u2, u3, u4, u5, u6, u7 = (
                        e[0:vt, :] for e in eb)
                    Ac = [A[r][t][0:vt, cb:cb + XC] for r in range(3)]
                    Dc = [D[r][t][0:vt, cb:cb + XC] for r in range(3)]
                    # e = a0*A0 + a1*A1 + a2*A2 + (a3 + 1e-4)
                    nc.vector.tensor_mul(out=u0, in0=a0, in1=Ac[0])
                    nc.vector.tensor_mul(out=u1, in0=a1, in1=Ac[1])
                    nc.vector.tensor_add(out=u0, in0=u0, in1=u1)
                    nc.vector.tensor_mul(out=u2, in0=a2, in1=Ac[2])
                    nc.vector.tensor_scalar_add(u3, a3, 1e-4)
                    nc.vector.tensor_add(out=u2, in0=u2, in1=u3)
                    nc.vector.tensor_add(out=u0, in0=u0, in1=u2)  # u0 = e
                    # d = a0*D0 + a1*D1 + a2*D2 + 1e-4
                    nc.vector.tensor_mul(out=u1, in0=a0, in1=Dc[0])
                    nc.vector.tensor_mul(out=u2, in0=a1, in1=Dc[1])
                    nc.vector.tensor_add(out=u1, in0=u1, in1=u2)
                    nc.vector.tensor_mul(out=u2, in0=a2, in1=Dc[2])
                    nc.vector.tensor_add(out=u1, in0=u1, in1=u2)
                    nc.vector.tensor_scalar_add(u2, u1, 1e-4)  # u2 = d
                    # num = a0*s3 + a1*s5 + a2*s7 + a3*feat + d*feat_init
                    nc.vector.tensor_mul(out=u3, in0=a0,
                                         in1=ps[0][ci][0:vt, :])
                    nc.vector.tensor_mul(out=u4, in0=a1,
                                         in1=ps[1][ci][0:vt, :])
                    nc.vector.tensor_add(out=u3, in0=u3, in1=u4)
                    nc.vector.tensor_mul(out=u4, in0=a2,
                                         in1=ps[2][ci][0:vt, :])
                    fc_c = fc[0:vt, cb:cb + XC]
                    nc.vector.tensor_mul(out=u5, in0=a3, in1=fc_c)
                    nc.vector.tensor_add(out=u4, in0=u4, in1=u5)
                    nc.vector.tensor_mul(out=u5, in0=u2,
                                         in1=fi_out[t][0:vt, cb:cb + XC])
                    nc.vector.tensor_add(out=u3, in0=u3, in1=u4)
                    nc.vector.tensor_add(out=u3, in0=u3, in1=u5)  # num
                    # r = 1/e: ACT table recip + one Newton step
                    act_recip(nc, u6, u0)
                    nc.vector.tensor_mul(out=u4, in0=u0, in1=u6)
                    nc.scalar.activation(out=u4, in_=u4, func=AF.Copy,
                                         bias=2.0, scale=-1.0)
                    nc.vector.tensor_mul(out=u6, in0=u6, in1=u4)
                    nc.vector.tensor_mul(out=u7, in0=u3, in1=u6)
                    to = tmp_out[ci]
                    nc.vector.tensor_mul(out=to[0:vt, :],
                                         in0=OM[t][0:vt, cb:cb + XC],
                                         in1=u7)
                    nc.vector.tensor_add(out=to[0:vt, :],
                                         in0=to[0:vt, :],
                                         in1=FF[t][0:vt, cb:cb + XC])
                    nc.sync.dma_start(
                        out=ft[t][3:3 + vt, 3 + cb:3 + cb + XC],
                        in_=to[0:vt, :])
            # seams between tiles (new feat values)
            nc.sync.dma_start(out=ft[1][0:3, :], in_=ft[0][122:125, :])
            nc.sync.dma_start(out=ft[0][125:128, :], in_=ft[1][3:6, :])
            nc.sync.dma_start(out=ft[2][0:3, :], in_=ft[1][122:125, :])
            nc.sync.dma_start(out=ft[1][125:128, :], in_=ft[2][3:6, :])

        # ================= output =================
        # bf16 staging halves the D2H bytes (DMA cannot convert dtypes);
        # copy at matching partition offset (DVE cannot shift partitions).
        obuf = [pool.tile([128, W], bf16, tag=f"ob{t}", name=f"ob{t}")
                for t in range(NT)]
        for t in range(NT):
            _, _, _, _, vt = tile_geom(t)
            r0 = TSTEP * t
            nc.vector.tensor_copy(out=obuf[t][:, :],
                                  in_=ft[t][:, 3:3 + W])
            nc.sync.dma_start(out=out[r0:r0 + vt, :],
                              in_=obuf[t][3:3 + vt, :])

    return nc


def fixup_waits(nc, cap=1):
    """Split >cap semaphore waits per instruction into prefix NoOps
    (this toolchain's codegen rejects multi-wait instructions)."""
    n_fixed = 0
    for f in nc.m.functions:
        for bb in f.blocks:
            insts = bb.instructions
            idx = 0
            changed = False
            while idx < len(insts):
                inst = insts[idx]
                si = inst.sync_info
                if si is None or si.on_wait is None or len(si.on_wait) <= cap:
                    idx += 1
                    continue
                waits = list(si.on_wait)
                head = waits[:-cap]
                for j in range(0, len(head), cap):
                    pre = bass_rust.InstNoOp(name=f"{inst.name}_wsplit{j}")
                    pre.engine = inst.engine
                    pre.debug = inst.debug
                    psi = copy.deepcopy(si)
                    psi.on_wait = head[j:j + cap]
                    psi.on_update = []
                    pre.sync_info = psi
                    insts.insert(idx, pre)
                    idx += 1
                si2 = inst.sync_info
                si2.on_wait = waits[-cap:]
                inst.sync_info = si2
                n_fixed += 1
                changed = True
                idx += 1
            if changed:
                bb.instructions = insts
    return n_fixed


_CACHE = {}


def _get_nc(qdt, gdt, ddt):
    key = (qdt, gdt, ddt)
    if key not in _CACHE:
        nc = build_nc(prop_time=6, qdt=qdt, gdt=gdt, ddt=ddt)
        fixup_waits(nc)
        _CACHE[key] = nc
    return _CACHE[key]


def _np_dt(dt):
    return mybir.dt.np(dt)


_LUT_CACHE = {}
_BUF_CACHE = {}


def _buf(key, shape, dtype):
    b = _BUF_CACHE.get(key)
    if b is None or b.shape != shape or b.dtype != dtype:
        b = _BUF_CACHE[key] = np.empty(shape, dtype)
    return b


def _small_float_lut(np_dt):
    if np_dt not in _LUT_CACHE:
        hi16 = (np.arange(65536, dtype=np.uint32) << np.uint32(16)).view(
            np.float32)
        with np.errstate(invalid="ignore", over="ignore"):
            _LUT_CACHE[np_dt] = hi16.astype(np_dt)
    return _LUT_CACHE[np_dt]


def _to_small_float(a, np_dt, out=None):
    """f32 -> np_dt via a 64K LUT on the high 16 bits (3x faster than
    ml_dtypes astype on this single-core host; differs from astype by at
    most one ulp on ~5% of values, well inside the quantization noise)."""
    a = np.ascontiguousarray(np.asarray(a, dtype=np.float32))
    lut = _small_float_lut(np_dt)
    idx = a.reshape(-1).view(np.uint16)[1::2]
    if out is not None:
        out.reshape(-1)[:] = lut[idx]
        return out
    return lut[idx].reshape(a.shape)


def kernel(feat_init, guidance, dynamic, confidence, feat_fix,
           _trace=False, _qdt=None, _gdt=None, _ddt=None):
    import ml_dtypes
    qdt = _qdt or mybir.dt.bfloat16
    gdt = _gdt or mybir.dt.float8e3
    ddt = _ddt or mybir.dt.float8e3
    nc = _get_nc(qdt, gdt, ddt)
    B, _, H, Wf = feat_init.shape
    bf = ml_dtypes.bfloat16
    gnp = _np_dt(gdt)
    guidance = np.asarray(guidance)
    dynamic = np.asarray(dynamic)
    # staging buffers are reused across calls (allocation/page-fault cost
    # only); every byte is rewritten below on each call
    gd_cv = _buf("gd", (B, CH + 24, H, Wf), gnp)
    for b in range(B):
        _to_small_float(guidance[b], gnp, out=gd_cv[b, :CH])
        _to_small_float(dynamic[b], gnp, out=gd_cv[b, CH:])
    fi_cv = np.asarray(feat_init).astype(bf)
    cf_cv = np.asarray(confidence).astype(bf)
    ff_cv = np.asarray(feat_fix).astype(bf)
    in_maps = []
    for c in range(8):
        b, half = c // 2, c % 2
        r0 = 0 if half == 0 else H - R
        rows = slice(r0, r0 + R)
        aux = _buf(("aux", c), (RPAD + 2 * R, W), bf)
        aux[0:3] = 0
        aux[3:3 + R] = fi_cv[b, 0, rows, :]
        aux[3 + R:RPAD] = 0
        aux[RPAD:RPAD + R] = cf_cv[b, 0, rows, :]
        aux[RPAD + R:] = ff_cv[b, 0, rows, :]
        in_maps.append({
            "gd": gd_cv[b, :, rows, :],
            "aux": aux,
        })
    try:
        res = run_bass_kernel_spmd(nc, in_maps, core_ids=list(range(8)),
                                   trace=_trace)
    except ModuleNotFoundError:
        # NTFF profiling hook unavailable in this environment
        res = run_bass_kernel_spmd(nc, in_maps, core_ids=list(range(8)),
                                   trace=False)
    except Exception:
        # transient NRT_EXEC_UNIT_UNRECOVERABLE device wedges have been
        # observed on this fabric; one retry usually clears them
        import time
        time.sleep(2.0)
        res = run_bass_kernel_spmd(nc, in_maps, core_ids=list(range(8)),
                                   trace=False)
    outf = np.zeros((B, 1, H, W), np.float32)
    for c in range(8):
        b, half = c // 2, c % 2
        o = res.results[c]["out"]
        if half == 0:
            outf[b, 0, 0:240] = o[0:240]
        else:
            outf[b, 0, H - 240:H] = o[R - 240:R]
    if _trace:
        return outf, res
    return outf


# revision 37
# speedup vs baseline: 12.8058x; 1.2575x over previous
"""Trainium2 Bass kernel for nn_Dynamic_7x7_naivev2 (CSPN-style propagation).

Self-contained: shards the batch x H-halves across 8 NeuronCores with an
18-row shrinking halo (no inter-core communication), runs a Bass/Tile
kernel per core, and reassembles the full output.

guidance/dynamic ship as fp8(e3m4) in a single DRAM param and are
upconverted on-engine; fi/cf/ff ship as one stacked bf16 param; the S
shift matrices are generated on device; the boundary zero-halo is
materialized on device (per-tile pre-zeroed g buffers) instead of
host-side padded copies. All of this exists to minimize host->device
bytes and per-call host work, which dominate wall time here.
"""
import copy

import numpy as np

import jax

# Persistent XLA executable cache: the per-call pjit re-compile otherwise
# re-runs the NEFF verify/package step (~0.8 s) on every invocation.
jax.config.update("jax_compilation_cache_dir", "/tmp/jax_cache")
jax.config.update("jax_persistent_cache_min_compile_time_secs", 0)
jax.config.update("jax_persistent_cache_min_entry_size_bytes", 0)

import bass_rust
import concourse.bass as bass
import concourse.mybir as mybir
from concourse.bass_utils import run_bass_kernel_spmd
from concourse.tile import TileContext
from contextlib import ExitStack


AF = mybir.ActivationFunctionType

R = 258          # local rows per shard
RPAD = 272       # padded DRAM rows for fi: 3 zero + 258 data + 11 zero
W = 640
X = 648          # q/feat tile cols (3 zero margin each side + 2 pad)
NT = 3           # row tiles
TSTEP = 122      # output rows per tile
CH = 48
XC = 320         # x chunk width (psum free dim)
CHUNKS = (0, 320)  # output col bases (global cols)

# (dy, dx) per guidance channel, ring 0 = 3x3 (ch 0:8), 1 = 5x5 (8:24),
# 2 = 7x7 (24:48). Derived numerically from the reference conv.
OFFS = [(1, 1), (1, 0), (1, -1), (0, 1), (0, -1), (-1, 1), (-1, 0), (-1, -1),
        (2, 2), (2, 1), (2, 0), (2, -1), (2, -2), (1, 2), (1, -2), (0, 2),
        (0, -2), (-1, 2), (-1, -2), (-2, 2), (-2, 1), (-2, 0), (-2, -1),
        (-2, -2),
        (3, 3), (3, 2), (3, 1), (3, 0), (3, -1), (3, -2), (3, -3), (2, 3),
        (2, -3), (1, 3), (1, -3), (0, 3), (0, -3), (-1, 3), (-1, -3),
        (-2, 3), (-2, -3), (-3, 3), (-3, 2), (-3, 1), (-3, 0), (-3, -1),
        (-3, -2), (-3, -3)]
RING_RANGES = ((0, 8), (8, 24), (24, 48))


def dy_groups(c0, c1):
    """Channels of a ring grouped by row shift dy: [(dy, [(ch, dx), ...])]."""
    g = {}
    for ch in range(c0, c1):
        dy, dx = OFFS[ch]
        g.setdefault(dy, []).append((ch, dx))
    return sorted(g.items())


def tile_geom(t):
    """(base_row, first_valid_part, end_valid_part, q_extent, valid_out)"""
    base = TSTEP * t - 3
    lo = max(0, -base)
    hi = min(128, R - base)
    qhi = min(128, hi + 7)
    vt = min(TSTEP, R - TSTEP * t)
    return base, lo, hi, qhi, vt


def act_recip(nc, out, in_):
    """scalar-engine Reciprocal, bypassing the accuracy guard (we Newton-refine)."""
    eng = nc.scalar
    return eng.add_instruction(
        mybir.InstActivation(
            name=nc.get_next_instruction_name(),
            func=AF.Reciprocal,
            ins=[eng.lower_ap(in_),
                 mybir.ImmediateValue(dtype=mybir.dt.float32, value=0.0),
                 mybir.ImmediateValue(dtype=mybir.dt.float32, value=1.0),
                 mybir.ImmediateValue(dtype=mybir.dt.float32, value=0.0)],
            outs=[eng.lower_ap(out)],
        )
    )


def build_nc(prop_time=6, qdt=mybir.dt.bfloat16, gdt=mybir.dt.float8e3,
             ddt=mybir.dt.float8e3):
    nc = bass.Bass()
    f32 = mybir.dt.float32

    bf16 = mybir.dt.bfloat16
    assert gdt == ddt, "g and dyn share one DRAM param"
    # planes 0:48 = guidance channels, 48:72 = dynamic planes (4*it + c)
    g_in = nc.declare_dram_parameter("gd", [CH + 4 * prop_time, R, W], gdt,
                                     isOutput=False)
    dyn_plane = CH
    # rows 0:272 = fi (3-row zero pad + 258 + 11 pad), 272:530 = cf,
    # 530:788 = ff — one bf16 param, one host->device put.
    AUX_CF = RPAD
    AUX_FF = RPAD + R
    aux_in = nc.declare_dram_parameter("aux", [RPAD + 2 * R, W], bf16,
                                       isOutput=False)
    out = nc.declare_dram_parameter("out", [R, W], bf16, isOutput=True)

    with ExitStack() as ctx:
        tc = ctx.enter_context(TileContext(nc))
        pool = ctx.enter_context(tc.tile_pool(name="main", bufs=1))
        pspool = ctx.enter_context(
            tc.tile_pool(name="ps", bufs=1, space="PSUM"))

        # ---- fixed tiles ----
        # S[dyi][k, j] = 1 iff k == j + dyi — generated on device, no input.
        S = [pool.tile([128, TSTEP], qdt, tag=f"S{i}", name=f"S{i}") for i in range(7)]
        for i in range(7):
            nc.gpsimd.memset(S[i][:], 1.0)
            nc.gpsimd.affine_select(
                out=S[i][:], in_=S[i][:],
                compare_op=mybir.AluOpType.is_equal, fill=0.0,
                base=-i, channel_multiplier=1, pattern=[[-1, TSTEP]])

        ft = [pool.tile([128, X], f32, tag=f"ft{t}", name=f"ft{t}") for t in range(NT)]
        fi_out = [pool.tile([TSTEP, W], f32, tag=f"fio{t}", name=f"fio{t}") for t in range(NT)]
        OM = [pool.tile([TSTEP, W], f32, tag=f"om{t}", name=f"om{t}") for t in range(NT)]
        FF = [pool.tile([TSTEP, W], f32, tag=f"ffp{t}", name=f"ffp{t}") for t in range(NT)]
        A = [[pool.tile([TSTEP, W], f32, tag=f"A{r}{t}", name=f"A{r}{t}") for t in range(NT)]
             for r in range(3)]
        D = [[pool.tile([TSTEP, W], f32, tag=f"D{r}{t}", name=f"D{r}{t}") for t in range(NT)]
             for r in range(3)]

        NG = 6
        NQ = 4
        # per-tile g buffers: boundary partitions stay zero from the initial
        # memset (loads never touch them), giving the halo for free.
        gb = [[pool.tile([128, W], gdt, tag=f"gb{t}_{i}", name=f"gb{t}_{i}")
               for i in range(NG)] for t in range(NT)]
        qb = [pool.tile([128, X], qdt, tag=f"qb{i}", name=f"qb{i}") for i in range(NQ)]
        fco = [pool.tile([TSTEP, W], f32, tag=f"fco{i}", name=f"fco{i}") for i in range(2)]
        dynb = [pool.tile([TSTEP, 4 * W], ddt, tag=f"dynb{i}", name=f"dynb{i}")
                for i in range(2)]
        attb = [pool.tile([TSTEP, 4 * W], f32, tag=f"attb{i}", name=f"attb{i}")
                for i in range(2)]
        cfb = pool.tile([TSTEP, W], bf16, tag="cfb", name="cfb")
        ffb = pool.tile([TSTEP, W], bf16, tag="ffb", name="ffb")
        fstage = pool.tile([128, W], bf16, tag="fstage", name="fstage")
        fstage2 = pool.tile([TSTEP, W], bf16, tag="fstage2", name="fstage2")
        sgn = pool.tile([TSTEP, W], f32, tag="sgn", name="sgn")
        fxb = pool.tile([TSTEP, W], f32, tag="fxb", name="fxb")
        tmp_out = [pool.tile([TSTEP, XC], f32, tag=f"tout{i}", name=f"tout{i}")
                   for i in range(2)]
        NE = 8
        eb = [pool.tile([TSTEP, XC], f32, tag=f"eb{i}", name=f"eb{i}") for i in range(NE)]
        NU = 2
        ub = [pool.tile([128, X], qdt, tag=f"ub{i}", name=f"ub{i}") for i in range(NU)]
        u_ctr = [0]

        for t in range(NT):
            nc.vector.memset(ft[t][:], 0.0)
            for i in range(NG):
                nc.vector.memset(gb[t][i][:], 0.0)
        for i in range(NQ):
            nc.vector.memset(qb[i][:], 0.0)

        def load_plane(dst, aux_base, t):
            _, _, _, _, vt = tile_geom(t)
            r0 = aux_base + TSTEP * t
            nc.sync.dma_start(out=dst[0:vt, :], in_=aux_in[r0:r0 + vt, :])

        def load_g_tile(dst, ch, t):
            """partition p of tile t <-> local row 122t - 3 + p; rows outside
            [0, R) stay zero in dst from the init memset."""
            base, _, _, qhi, _ = tile_geom(t)
            p0 = max(0, -base)
            n = min(R, base + qhi) - (base + p0)
            nc.sync.dma_start(out=dst[p0:p0 + n, :],
                              in_=g_in[ch, base + p0:base + p0 + n, :])

        def psum_tiles():
            return [[pspool.tile([TSTEP, XC], f32, tag=f"ps{r}{c}", name=f"ps{r}{c}")
                     for c in range(2)] for r in range(3)]

        def ring_sweep(t, ps, prep):
            """48-channel sweep: load g, prep(qq, g, qhi), then the ring
            shift-matmuls of qq into ps[ring][chunk]."""
            base, lo, hi, qhi, vt = tile_geom(t)
            for ri, (c0, c1) in enumerate(RING_RANGES):
                for ch in range(c0, c1):
                    g = gb[t][ch % NG]
                    load_g_tile(g, ch, t)
                    qq = qb[ch % NQ]
                    prep(qq, g, qhi)
                    dy, dx = OFFS[ch]
                    first = ch == c0
                    last = ch == c1 - 1
                    for ci, cb in enumerate(CHUNKS):
                        nc.tensor.matmul(
                            ps[ri][ci][:],
                            lhsT=S[dy + 3][:],
                            rhs=qq[:, cb + 3 + dx:cb + 3 + dx + XC],
                            start=first, stop=last)

        def ring_sweep_grouped(t, ps, prep):
            """Like ring_sweep, but channels sharing a row shift dy are
            pre-summed (with their column shifts) on the vector engine into
            one u tile, so each ring needs one matmul pair per dy instead
            of per channel (96 -> ~30 PE instructions per sweep-tile)."""
            base, lo, hi, qhi, vt = tile_geom(t)
            for ri, (c0, c1) in enumerate(RING_RANGES):
                groups = dy_groups(c0, c1)
                for gi, (dy, members) in enumerate(groups):
                    u = ub[u_ctr[0] % NU]
                    u_ctr[0] += 1
                    for mi, (ch, dx) in enumerate(members):
                        g = gb[t][ch % NG]
                        load_g_tile(g, ch, t)
                        qq = qb[ch % NQ]
                        prep(qq, g, qhi)
                        if mi == 0:
                            nc.vector.tensor_copy(
                                out=u[0:qhi, 3:3 + W],
                                in_=qq[0:qhi, 3 + dx:3 + dx + W])
                        else:
                            nc.vector.tensor_add(
                                out=u[0:qhi, 3:3 + W],
                                in0=u[0:qhi, 3:3 + W],
                                in1=qq[0:qhi, 3 + dx:3 + dx + W])
                    first = gi == 0
                    last = gi == len(groups) - 1
                    for ci, cb in enumerate(CHUNKS):
                        nc.tensor.matmul(
                            ps[ri][ci][:],
                            lhsT=S[dy + 3][:],
                            rhs=u[:, cb + 3:cb + 3 + XC],
                            start=first, stop=last)

        # ================= setup =================
        for t in range(NT):
            base, lo, hi, qhi, vt = tile_geom(t)
            n = min(128, R + 3 - TSTEP * t)
            nc.sync.dma_start(out=fstage[0:n, :],
                              in_=aux_in[TSTEP * t:TSTEP * t + n, :])
            nc.vector.tensor_copy(out=ft[t][0:n, 3:3 + W],
                                  in_=fstage[0:n, :])
            r0 = TSTEP * t
            nc.sync.dma_start(out=fstage2[0:vt, :],
                              in_=aux_in[r0 + 3:r0 + 3 + vt, :])
            nc.vector.tensor_copy(out=fi_out[t][0:vt, :],
                                  in_=fstage2[0:vt, :])
            load_plane(cfb, AUX_CF, t)
            load_plane(ffb, AUX_FF, t)
            nc.scalar.sign(out=sgn[0:vt], in_=ffb[0:vt])
            nc.vector.tensor_mul(out=fxb[0:vt, :], in0=sgn[0:vt, :],
                                 in1=cfb[0:vt, :])
            nc.scalar.activation(out=OM[t][0:vt], in_=fxb[0:vt], func=AF.Copy,
                                 bias=1.0, scale=-1.0)
            nc.vector.tensor_mul(out=FF[t][0:vt, :], in0=fxb[0:vt, :],
                                 in1=ffb[0:vt, :])

        # aff sums at output rows: A = ring sums of |g|; D = A - sums of g
        # = ring sums of (|g| - g) = 2 * ring sums of relu(-g). Both are
        # sums of nonnegative terms, so the grouped (bf16-presummed) sweep
        # is cancellation-free for each.
        for t in range(NT):
            base, lo, hi, qhi, vt = tile_geom(t)

            def prep_abs(qq, g, qh):
                nc.scalar.activation(out=qq[0:qh, 3:3 + W], in_=g[0:qh, :],
                                     func=AF.Abs)

            psA = psum_tiles()
            ring_sweep_grouped(t, psA, prep_abs)
            for ri in range(3):
                for ci, cb in enumerate(CHUNKS):
                    nc.scalar.copy(out=A[ri][t][0:vt, cb:cb + XC],
                                   in_=psA[ri][ci][0:vt, :])

            def prep_negrelu(qq, g, qh):
                nc.scalar.activation(out=qq[0:qh, 3:3 + W], in_=g[0:qh, :],
                                     func=AF.Relu, scale=-1.0)

            psB = psum_tiles()
            ring_sweep_grouped(t, psB, prep_negrelu)
            for ri in range(3):
                for ci, cb in enumerate(CHUNKS):
                    nc.scalar.activation(out=D[ri][t][0:vt, cb:cb + XC],
                                         in_=psB[ri][ci][0:vt, :],
                                         func=AF.Copy, scale=2.0)

        # ================= iterations =================
        for it in range(prop_time):
            for t in range(NT):
                base, lo, hi, qhi, vt = tile_geom(t)
                fc = fco[t % 2]
                nc.sync.dma_start(out=fc[0:vt, :],
                                  in_=ft[t][3:3 + vt, 3:3 + W])
                dynt = dynb[t % 2]
                att = attb[t % 2]
                r0 = TSTEP * t
                for c in range(4):
                    nc.sync.dma_start(
                        out=dynt[0:vt, c * W:(c + 1) * W],
                        in_=g_in[dyn_plane + 4 * it + c, r0:r0 + vt, :])
                nc.scalar.activation(out=att[0:vt, :], in_=dynt[0:vt, :],
                                     func=AF.Sigmoid)

                def prep_mul(qq, g, qh, t=t):
                    nc.vector.tensor_mul(out=qq[0:qh, 3:3 + W],
                                         in0=ft[t][0:qh, 3:3 + W],
                                         in1=g[0:qh, :])

                ps = psum_tiles()
                ring_sweep_grouped(t, ps, prep_mul)

                for ci, cb in enumerate(CHUNKS):
                    a0 = att[0:vt, 0 * W + cb:0 * W + cb + XC]
                    a1 = att[0:vt, 1 * W + cb:1 * W + cb + XC]
                    a2 = att[0:vt, 2 * W + cb:2 * W + cb + XC]
                    a3 = att[0:vt, 3 * W + cb:3 * W + cb + XC]
                    u0, u1, u2, u3, u4, u5, u6, u7 = (
                        e[0:vt, :] for e in eb)
                    Ac = [A[r][t][0:vt, cb:cb + XC] for r in range(3)]
                    Dc = [D[r][t][0:vt, cb:cb + XC] for r in range(3)]
                    # e = a0*A0 + a1*A1 + a2*A2 + (a3 + 1e-4)
                    nc.vector.tensor_mul(out=u0, in0=a0, in1=Ac[0])
                    nc.vector.tensor_mul(out=u1, in0=a1, in1=Ac[1])
                    nc.vector.tensor_add(out=u0, in0=u0, in1=u1)
                    nc.vector.tensor_mul(out=u2, in0=a2, in1=Ac[2])
                    nc.vector.tensor_scalar_add(u3, a3, 1e-4)
                    nc.vector.tensor_add(out=u2, in0=u2, in1=u3)
                    nc.vector.tensor_add(out=u0, in0=u0, in1=u2)  # u0 = e
                    # d = a0*D0 + a1*D1 + a2*D2 + 1e-4
                    nc.vector.tensor_mul(out=u1, in0=a0, in1=Dc[0])
                    nc.vector.tensor_mul(out=u2, in0=a1, in1=Dc[1])
                    nc.vector.tensor_add(out=u1, in0=u1, in1=u2)
                    nc.vector.tensor_mul(out=u2, in0=a2, in1=Dc[2])
                    nc.vector.tensor_add(out=u1, in0=u1, in1=u2)
                    nc.vector.tensor_scalar_add(u2, u1, 1e-4)  # u2 = d
                    # num = a0*s3 + a1*s5 + a2*s7 + a3*feat + d*feat_init
                    nc.vector.tensor_mul(out=u3, in0=a0,
                                         in1=ps[0][ci][0:vt, :])
                    nc.vector.tensor_mul(out=u4, in0=a1,
                                         in1=ps[1][ci][0:vt, :])
                    nc.vector.tensor_add(out=u3, in0=u3, in1=u4)
                    nc.vector.tensor_mul(out=u4, in0=a2,
                                         in1=ps[2][ci][0:vt, :])
                    fc_c = fc[0:vt, cb:cb + XC]
                    nc.vector.tensor_mul(out=u5, in0=a3, in1=fc_c)
                    nc.vector.tensor_add(out=u4, in0=u4, in1=u5)
                    nc.vector.tensor_mul(out=u5, in0=u2,
                                         in1=fi_out[t][0:vt, cb:cb + XC])
                    nc.vector.tensor_add(out=u3, in0=u3, in1=u4)
                    nc.vector.tensor_add(out=u3, in0=u3, in1=u5)  # num
                    # r = 1/e: ACT table recip + one Newton step
                    act_recip(nc, u6, u0)
                    nc.vector.tensor_mul(out=u4, in0=u0, in1=u6)
                    nc.scalar.activation(out=u4, in_=u4, func=AF.Copy,
                                         bias=2.0, scale=-1.0)
                    nc.vector.tensor_mul(out=u6, in0=u6, in1=u4)
                    nc.vector.tensor_mul(out=u7, in0=u3, in1=u6)
                    to = tmp_out[ci]
                    nc.vector.tensor_mul(out=to[0:vt, :],
                                         in0=OM[t][0:vt, cb:cb + XC],
                                         in1=u7)
                    nc.vector.tensor_add(out=to[0:vt, :],
                                         in0=to[0:vt, :],
                                         in1=FF[t][0:vt, cb:cb + XC])
                    nc.sync.dma_start(
                        out=ft[t][3:3 + vt, 3 + cb:3 + cb + XC],
                        in_=to[0:vt, :])
            # seams between tiles (new feat values)
            nc.sync.dma_start(out=ft[1][0:3, :], in_=ft[0][122:125, :])
            nc.sync.dma_start(out=ft[0][125:128, :], in_=ft[1][3:6, :])
            nc.sync.dma_start(out=ft[2][0:3, :], in_=ft[1][122:125, :])
            nc.sync.dma_start(out=ft[1][125:128, :], in_=ft[2][3:6, :])

        # ================= output =================
        # bf16 staging halves the D2H bytes (DMA cannot convert dtypes);
        # copy at matching partition offset (DVE cannot shift partitions).
        obuf = [pool.tile([128, W], bf16, tag=f"ob{t}", name=f"ob{t}")
                for t in range(NT)]
        for t in range(NT):
            _, _, _, _, vt = tile_geom(t)
            r0 = TSTEP * t
            nc.vector.tensor_copy(out=obuf[t][:, :],
                                  in_=ft[t][:, 3:3 + W])
            nc.sync.dma_start(out=out[r0:r0 + vt, :],
                              in_=obuf[t][3:3 + vt, :])

    return nc


def fixup_waits(nc, cap=1):
    """Split >cap semaphore waits per instruction into prefix NoOps
    (this toolchain's codegen rejects multi-wait instructions)."""
    n_fixed = 0
    for f in nc.m.functions:
        for bb in f.blocks:
            insts = bb.instructions
            idx = 0
            changed = False
            while idx < len(insts):
                inst = insts[idx]
                si = inst.sync_info
                if si is None or si.on_wait is None or len(si.on_wait) <= cap:
                    idx += 1
                    continue
                waits = list(si.on_wait)
                head = waits[:-cap]
                for j in range(0, len(head), cap):
                    pre = bass_rust.InstNoOp(name=f"{inst.name}_wsplit{j}")
                    pre.engine = inst.engine
                    pre.debug = inst.debug
                    psi = copy.deepcopy(si)
                    psi.on_wait = head[j:j + cap]
                    psi.on_update = []
                    pre.sync_info = psi
                    insts.insert(idx, pre)
                    idx += 1
                si2 = inst.sync_info
                si2.on_wait = waits[-cap:]
                inst.sync_info = si2
                n_fixed += 1
                changed = True
                idx += 1
            if changed:
                bb.instructions = insts
    return n_fixed


_CACHE = {}


def _get_nc(qdt, gdt, ddt):
    key = (qdt, gdt, ddt)
    if key not in _CACHE:
        nc = build_nc(prop_time=6, qdt=qdt, gdt=gdt, ddt=ddt)
        fixup_waits(nc)
        _CACHE[key] = nc
    return _CACHE[key]


def _np_dt(dt):
    return mybir.dt.np(dt)


_LUT_CACHE = {}
_BUF_CACHE = {}


def _buf(key, shape, dtype):
    b = _BUF_CACHE.get(key)
    if b is None or b.shape != shape or b.dtype != dtype:
        b = _BUF_CACHE[key] = np.empty(shape, dtype)
    return b


def _small_float_lut(np_dt):
    if np_dt not in _LUT_CACHE:
        hi16 = (np.arange(65536, dtype=np.uint32) << np.uint32(16)).view(
            np.float32)
        with np.errstate(invalid="ignore", over="ignore"):
            _LUT_CACHE[np_dt] = hi16.astype(np_dt)
    return _LUT_CACHE[np_dt]


try:
    import numba

    @numba.njit(cache=False)
    def _lut_gather(src_u16, lut_u8, out_u8):
        for i in range(out_u8.size):
            out_u8[i] = lut_u8[src_u16[i]]
except ImportError:  # pragma: no cover
    _lut_gather = None


def _to_small_float(a, np_dt, out=None):
    """f32 -> np_dt via a 64K LUT on the high 16 bits (6x faster than
    ml_dtypes astype on this single-core host via the numba loop; differs
    from astype by at most one ulp on ~5% of values, well inside the
    quantization noise)."""
    a = np.ascontiguousarray(np.asarray(a, dtype=np.float32))
    lut = _small_float_lut(np_dt)
    idx = a.reshape(-1).view(np.uint16)[1::2]
    if out is None:
        out = np.empty(a.shape, np_dt)
    if _lut_gather is not None:
        _lut_gather(idx, lut.view(np.uint8), out.reshape(-1).view(np.uint8))
    else:
        out.reshape(-1)[:] = lut[idx]
    return out


def kernel(feat_init, guidance, dynamic, confidence, feat_fix,
           _trace=False, _qdt=None, _gdt=None, _ddt=None):
    import ml_dtypes
    qdt = _qdt or mybir.dt.bfloat16
    gdt = _gdt or mybir.dt.float8e3
    ddt = _ddt or mybir.dt.float8e3
    nc = _get_nc(qdt, gdt, ddt)
    B, _, H, Wf = feat_init.shape
    bf = ml_dtypes.bfloat16
    gnp = _np_dt(gdt)
    guidance = np.asarray(guidance)
    dynamic = np.asarray(dynamic)
    # staging buffers are reused across calls (allocation/page-fault cost
    # only); every byte is rewritten below on each call
    gd_cv = _buf("gd", (B, CH + 24, H, Wf), gnp)
    for b in range(B):
        _to_small_float(guidance[b], gnp, out=gd_cv[b, :CH])
        _to_small_float(dynamic[b], gnp, out=gd_cv[b, CH:])
    fi_cv = np.asarray(feat_init).astype(bf)
    cf_cv = np.asarray(confidence).astype(bf)
    ff_cv = np.asarray(feat_fix).astype(bf)
    in_maps = []
    for c in range(8):
        b, half = c // 2, c % 2
        r0 = 0 if half == 0 else H - R
        rows = slice(r0, r0 + R)
        aux = _buf(("aux", c), (RPAD + 2 * R, W), bf)
        aux[0:3] = 0
        aux[3:3 + R] = fi_cv[b, 0, rows, :]
        aux[3 + R:RPAD] = 0
        aux[RPAD:RPAD + R] = cf_cv[b, 0, rows, :]
        aux[RPAD + R:] = ff_cv[b, 0, rows, :]
        in_maps.append({
            "gd": gd_cv[b, :, rows, :],
            "aux": aux,
        })
    try:
        res = run_bass_kernel_spmd(nc, in_maps, core_ids=list(range(8)),
                                   trace=_trace)
    except ModuleNotFoundError:
        # NTFF profiling hook unavailable in this environment
        res = run_bass_kernel_spmd(nc, in_maps, core_ids=list(range(8)),
                                   trace=False)
    except Exception:
        # transient NRT_EXEC_UNIT_UNRECOVERABLE device wedges have been
        # observed on this fabric; one retry usually clears them
        import time
        time.sleep(2.0)
        res = run_bass_kernel_spmd(nc, in_maps, core_ids=list(range(8)),
                                   trace=False)
    outf = np.zeros((B, 1, H, W), np.float32)
    for c in range(8):
        b, half = c // 2, c % 2
        o = res.results[c]["out"]
        if half == 0:
            outf[b, 0, 0:240] = o[0:240]
        else:
            outf[b, 0, H - 240:H] = o[R - 240:R]
    if _trace:
        return outf, res
    return outf
